# revision 38
# baseline (speedup 1.0000x reference)
"""EvolvingGNN (gnn_message_passing) kernel for 8 Trainium2 NeuronCores.

Self-contained: takes the full inputs of setup_inputs(), returns the full
[E] float32 logits.  See kernel() at the bottom.

Structure:
  host: dinv-scaled x4d (so messages gather straight from the input and the
        device-side phase-1 y materialization disappears -- segment_sum
        commutes with the @W5 right-matmul, applied per window after
        aggregation); balanced dst relabeling node -> (core, window,
        partition) equalizes per-(core,window,chunk) message counts across
        cores (gather capacities are max-over-cores); edge bucketing by
        (dst-core, dst-window, src-chunk) for P2 and (src-chunk, dst-chunk)
        cells for P4, with UV rows in a partition-major permuted layout so
        the UV_shard DMA writes are per-partition contiguous.
  P0: LSTM -> W5 (tiny).
  P2: gather x4d[src] (dma_gather spread over all 4 SWDGE queues -- the
      per-queue descriptor pacing is the bottleneck resource), cast bf16 on
      ScalarE, one-hot (iota==dl) scatter-add via TensorE into PSUM, then
      xl^T = relu(W5^T @ Sx) per window, kept in SBUF.
  P3: per window U||V = dinv * (xl @ [W1a|W1b]) bf16 packed in 256B rows,
      single AllGather bitcast to f32 (halves collective element count).
  P4: per edge: gather UV[src], UV[dst] (256B rows, greedy queue balance),
      U+V presummed on DVE, + ea@W1cb in PSUM, hid = relu(.),
      logits = hid @ w2 + b2.  First-superslab index tiles are preloaded
      before the collective (everything issued after it waits on it).
"""


import heapq

import numpy as np
import ml_dtypes

import concourse.bacc as bacc
import concourse.tile as tile
from concourse import mybir
from concourse.bass_utils import run_bass_kernel_spmd

F32 = mybir.dt.float32
BF16 = mybir.dt.bfloat16
I16 = mybir.dt.int16
AF = mybir.ActivationFunctionType
OP = mybir.AluOpType
BF_NP = np.dtype(ml_dtypes.bfloat16)

D = 64   # input/lstm dim
H = 64   # hidden dim
FE = 16  # edge feature dim
C = 8    # cores
NQ = 4   # SWDGE queues


class Cfg:
    def __init__(self, N, E, gw=5, ssl=32, sl=8):
        self.N, self.E = N, E
        self.NP = -(-N // (C * 128)) * C * 128      # padded nodes
        self.SHARD = self.NP // C                    # nodes per core
        self.WPC = self.SHARD // 128                 # windows per core
        self.CS = self.NP // 4                       # gather chunk rows
        assert self.CS < 32768, "chunk exceeds int16 gather range"
        assert E % C == 0
        self.EPC = E // C                            # edges per core
        self.GW = gw                                 # windows per ph2 gather group
        self.SSL = ssl                               # ph4 superslab tiles
        self.SL = sl                                 # ph4 inner slab tiles
        self.groups = [range(s, min(s + gw, self.WPC))
                       for s in range(0, self.WPC, gw)]


def _wrap16(arr_i16):
    """[S] -> [128, S/16] wrapped in 16 partitions, replicated x8."""
    w = arr_i16.reshape(-1, 16).T  # [16, S/16]
    return np.ascontiguousarray(np.tile(w, (8, 1)))


def _ranks_after_sort(sorted_keys):
    """rank of each element within its (contiguous) key run."""
    n = len(sorted_keys)
    if n == 0:
        return np.zeros(0, np.int64)
    change = np.r_[True, sorted_keys[1:] != sorted_keys[:-1]]
    starts = np.where(change)[0]
    return np.arange(n) - np.repeat(starts, np.diff(np.r_[starts, n]))


def prep(inputs, cfg):
    """Host preprocessing. Returns (in_maps, static, meta)."""
    N, E, NP, CS = cfg.N, cfg.E, cfg.NP, cfg.CS
    WPC, SHARD = cfg.WPC, cfg.SHARD

    ei = np.asarray(inputs["edge_index"])
    src = ei[0].astype(np.int64)
    dst = ei[1].astype(np.int64)
    loops = np.arange(N, dtype=np.int64)
    srcA = np.concatenate([src, loops])
    dstA = np.concatenate([dst, loops])

    deg = np.bincount(dstA, minlength=NP).astype(np.float32)
    deg[deg == 0] = 1.0
    dinv = (deg ** -0.5).astype(np.float32)

    # dinv-scaled input features (message payload), padded, ORIGINAL node order
    xs = np.asarray(inputs["xs"], dtype=np.float32)
    x4d = np.zeros((NP, D), np.float32)
    x4d[:N] = dinv[:N, None] * xs[-1]

    q = srcA // CS

    # ---- balanced dst relabeling: node -> (core, window, partition) so that
    # per-(core,window,chunk) message counts are even across cores (the
    # gather capacities are max-over-cores; balance cuts padded slots ~13%)
    tot = np.bincount(dstA, minlength=NP)
    order_n = np.argsort(-tot, kind="stable")
    NB = C * WPC
    heap = [(0, b) for b in range(NB)]
    heapq.heapify(heap)
    bucket_of = np.zeros(NP, np.int64)
    p_of = np.zeros(NP, np.int64)
    fill = np.zeros(NB, np.int64)
    for n in order_n:
        while True:
            load, b = heapq.heappop(heap)
            if fill[b] < 128:
                break
        bucket_of[n] = b
        p_of[n] = fill[b]
        fill[b] += 1
        if fill[b] < 128:
            heapq.heappush(heap, (load + int(tot[n]), b))
    core_of = bucket_of // WPC
    win_of = bucket_of % WPC
    # UV row of node n (partition-major within its shard)
    uvrow_of = core_of * SHARD + p_of * WPC + win_of
    dinv_sh_arr = np.zeros(NB * 128, np.float32)
    dinv_sh_arr[bucket_of * 128 + p_of] = dinv
    dinv_shard = [np.ascontiguousarray(
        dinv_sh_arr.reshape(C, WPC, 128)[c].T) for c in range(C)]

    # ---- phase 2 bucketing: key = (core, window, chunk) ----
    core = core_of[dstA]
    win = win_of[dstA]
    dloc = p_of[dstA].astype(np.float32)
    sloc = (srcA % CS).astype(np.int16)

    key = (core * WPC + win) * 4 + q
    order = np.argsort(key, kind="stable")
    skey = key[order]
    counts = np.bincount(key, minlength=C * WPC * 4).reshape(C, WPC, 4)
    capT = np.ceil(counts.max(axis=0) / 128).astype(np.int64)      # [WPC, 4] tiles

    # gather-stream order: (group, chunk, window) ; dl-stream order: (window, chunk)
    slots_wq = capT * 128
    g_off = np.zeros((WPC, 4), np.int64)     # slot offset in gather stream
    t_off = np.zeros((WPC, 4), np.int64)     # tile offset in dl stream
    pos = 0
    for g in cfg.groups:
        for qq in range(4):
            for w in g:
                g_off[w, qq] = pos
                pos += slots_wq[w, qq]
    total_slots2 = pos
    pos = 0
    for w in range(WPC):
        for qq in range(4):
            t_off[w, qq] = pos
            pos += capT[w, qq]
    total_tiles2 = pos

    ranks = _ranks_after_sort(skey)
    gpos = g_off[win[order], q[order]] + ranks                     # slot in gather stream
    dpos = t_off[win[order], q[order]] * 128 + ranks               # slot in dl stream
    corder = core[order]

    gidx_list, dl_list = [], []
    for c in range(C):
        m = corder == c
        garr = np.zeros(total_slots2, np.int16)
        garr[gpos[m]] = sloc[order][m]
        dlarr = np.full(total_tiles2 * 128, -1.0, np.float32)
        dlarr[dpos[m]] = dloc[order][m]
        gidx_list.append(_wrap16(garr))
        dl_list.append(np.ascontiguousarray(dlarr.reshape(-1, 128).T.astype(BF_NP)))

    # ---- phase 4 bucketing: per core (positional edge shard), key = (a, b) ----
    # UV rows live in the relabeled partition-major layout (uvrow_of)
    EPC = cfg.EPC
    perm_row = uvrow_of[src]
    perm_col = uvrow_of[dst]
    a_all = perm_row // CS
    b_all = perm_col // CS
    sl4 = (perm_row % CS).astype(np.int16)
    dl4 = (perm_col % CS).astype(np.int16)

    counts4 = np.zeros((C, 16), np.int64)
    key4 = (a_all * 4 + b_all)
    for c in range(C):
        counts4[c] = np.bincount(key4[c * EPC:(c + 1) * EPC], minlength=16)
    capE = np.ceil(counts4.max(axis=0) / 128).astype(np.int64)     # [16] tiles
    cell_off = np.r_[0, np.cumsum(capE)][:16]                       # tile offsets
    total_tiles4 = int(capE.sum())
    total_slots4 = total_tiles4 * 128

    ea = np.asarray(inputs["edge_attr"], dtype=np.float32)
    sidx_list, didx_list, eat_list, origmap = [], [], [], []
    for c in range(C):
        sl_ = slice(c * EPC, (c + 1) * EPC)
        k = key4[sl_]
        o = np.argsort(k, kind="stable")
        r = _ranks_after_sort(k[o])
        slotpos = cell_off[k[o]] * 128 + r
        sarr = np.zeros(total_slots4, np.int16)
        darr = np.zeros(total_slots4, np.int16)
        eat = np.zeros((17, total_slots4), np.float32)
        om = np.full(total_slots4, -1, np.int64)
        eids = np.arange(c * EPC, (c + 1) * EPC)[o]
        sarr[slotpos] = sl4[sl_][o]
        darr[slotpos] = dl4[sl_][o]
        eat[:16, slotpos] = ea[eids].T
        eat[16, slotpos] = 1.0
        om[slotpos] = eids
        sidx_list.append(_wrap16(sarr))
        didx_list.append(_wrap16(darr))
        eat_list.append(np.ascontiguousarray(eat.astype(BF_NP)))
        origmap.append(om)

    # ---- dense weights ----
    w_ihT = np.ascontiguousarray(np.asarray(inputs["w_ih"], np.float32).T)   # [64, 256]
    w_hhT = np.ascontiguousarray(np.asarray(inputs["w_hh"], np.float32).T)
    bstack = np.stack([np.asarray(inputs["b_ih"], np.float32),
                       np.asarray(inputs["b_hh"], np.float32)])              # [2, 256]
    ones2 = np.ones((2, D), np.float32)
    init_w = np.asarray(inputs["init_w"], np.float32)
    mlp_w1 = np.asarray(inputs["mlp_w1"], np.float32)
    W1ab = np.ascontiguousarray(
        np.hstack([mlp_w1[:H], mlp_w1[H:2 * H]]).astype(BF_NP))              # [64, 128]
    W1cb = np.ascontiguousarray(
        np.vstack([mlp_w1[2 * H:],
                   np.asarray(inputs["mlp_b1"], np.float32)[None]]).astype(BF_NP))  # [17, 64]
    w2b = np.ascontiguousarray(
        np.tile(np.asarray(inputs["mlp_w2"], np.float32).T, (128, 1)).astype(BF_NP))  # [128, 64]
    b2b = np.full((128, 1), np.asarray(inputs["mlp_b2"], np.float32)[0], np.float32)

    in_maps = []
    for c in range(C):
        in_maps.append(dict(
            x4d=x4d, dinvS=dinv_shard[c],
            gidx=gidx_list[c], dl_all=dl_list[c],
            sidx=sidx_list[c], didx=didx_list[c], ea_t=eat_list[c],
            w_ihT=w_ihT, w_hhT=w_hhT, bstack=bstack, ones2=ones2,
            init_w=init_w, W1ab=W1ab, W1cb=W1cb, w2b=w2b, b2b=b2b,
        ))

    static = dict(capT=capT, g_off=g_off, t_off=t_off,
                  total_slots2=total_slots2, total_tiles2=total_tiles2,
                  capE=capE, cell_off=cell_off,
                  total_slots4=total_slots4, total_tiles4=total_tiles4)
    meta = dict(origmap=origmap, total_tiles4=total_tiles4)
    return in_maps, static, meta


def unshard(results, meta, E):
    logits = np.zeros(E, np.float32)
    for c in range(C):
        out = np.asarray(results[c]["logits_out"])  # [128, T4]
        flat = out.T.reshape(-1)                    # slot j*128+p at [p, j] -> flat[j*128+p]
        om = meta["origmap"][c]
        m = om >= 0
        logits[om[m]] = flat[m]
    return logits


def build(cfg, static):
    NP, CS, WPC, SHARD = cfg.NP, cfg.CS, cfg.WPC, cfg.SHARD
    capT, g_off, t_off = static["capT"], static["g_off"], static["t_off"]
    S2, T2 = static["total_slots2"], static["total_tiles2"]
    capE, cell_off = static["capE"], static["cell_off"]
    S4, T4 = static["total_slots4"], static["total_tiles4"]

    nc = bacc.Bacc("TRN2", target_bir_lowering=False, num_devices=C,
                   num_swdge_queues=NQ, dynamic_dma_scratch_size=32768)

    # ---- params ----
    P = lambda name, shape, dt=F32: nc.declare_dram_parameter(name, list(shape), dt, isOutput=False)
    x4d = P("x4d", [NP, D])
    dinvS = P("dinvS", [128, WPC])
    gidx = P("gidx", [128, S2 // 16], I16)
    dl_all = P("dl_all", [128, T2], BF16)
    sidx = P("sidx", [128, S4 // 16], I16)
    didx = P("didx", [128, S4 // 16], I16)
    ea_t = P("ea_t", [17, S4], BF16)
    w_ihT = P("w_ihT", [D, 4 * D])
    w_hhT = P("w_hhT", [D, 4 * D])
    bstack = P("bstack", [2, 4 * D])
    ones2 = P("ones2", [2, D])
    init_w = P("init_w", [D, H])
    W1ab = P("W1ab", [H, 2 * H], BF16)
    W1cb = P("W1cb", [FE + 1, H], BF16)
    w2b = P("w2b", [128, H], BF16)
    b2b = P("b2b", [128, 1])
    logits_out = nc.declare_dram_parameter("logits_out", [128, T4], F32, isOutput=True)

    # ---- internal dram ----
    # UV is bf16 [*, 2H] data stored/collected as f32 [*, H] (pure bitcast):
    # halves the AllGather element count and keeps 256B gather elements.
    UV_shard = nc.dram_tensor("UV_shard", [SHARD, H], F32)
    UV_full = nc.dram_tensor("UV_full", [NP, H], F32, addr_space="Shared")

    iota_np = np.tile(np.arange(128, dtype=np.float32), (128, 1))
    iota_dram = nc.inline_tensor(iota_np, name="iota128")
    ident_dram = nc.inline_tensor(np.eye(128, dtype=np.float32), name="ident128")

    # per-queue DMA-completion sems for phase-4 prepare_only gathers
    qsems = [nc.alloc_semaphore(name=f"gsem{q}") for q in range(NQ)]

    with tile.TileContext(nc) as tc:
        with tc.tile_pool(name="persist", bufs=1) as pp:
            for s_ in qsems:
                nc.gpsimd.sem_clear(s_)
            iota_sb = pp.tile([128, 128], F32)
            nc.sync.dma_start(out=iota_sb[:], in_=iota_dram[:])
            iotab_sb = pp.tile([128, 128], BF16)
            nc.vector.tensor_copy(out=iotab_sb[:], in_=iota_sb[:])
            ident_sb = pp.tile([128, 128], F32)
            nc.sync.dma_start(out=ident_sb[:], in_=ident_dram[:])
            identb_sb = pp.tile([128, 128], BF16)
            nc.vector.tensor_copy(out=identb_sb[:], in_=ident_sb[:])
            w2b_sb = pp.tile([128, H], BF16)
            nc.sync.dma_start(out=w2b_sb[:], in_=w2b[:])
            b2b_sb = pp.tile([128, 1], F32)
            nc.sync.dma_start(out=b2b_sb[:], in_=b2b[:])
            W1ab_sb = pp.tile([H, 2 * H], BF16)
            nc.sync.dma_start(out=W1ab_sb[:], in_=W1ab[:])
            W1cb_sb = pp.tile([FE + 1, H], BF16)
            nc.sync.dma_start(out=W1cb_sb[:], in_=W1cb[:])
            dinv_sh = pp.tile([128, WPC], F32)
            nc.sync.dma_start(out=dinv_sh[:], in_=dinvS[:])
            # xl^T of this core's shard, bf16, lives in SBUF for all of P2/P3
            xts = pp.tile([H, SHARD], BF16)

            # ---- phase 0: LSTM -> W5 in w_sb [D, H]; bf16 copy in w5b ----
            w_sb = pp.tile([D, H], F32)
            w5b = pp.tile([D, H], BF16)
            nc.sync.dma_start(out=w_sb[:], in_=init_w[:])
            with (
                tc.tile_pool(name="lstm", bufs=1) as lp,
                tc.tile_pool(name="lstm_ps", bufs=2, space="PSUM") as lps,
            ):
                wih_sb = lp.tile([D, 4 * D], F32)
                nc.sync.dma_start(out=wih_sb[:], in_=w_ihT[:])
                whh_sb = lp.tile([D, 4 * D], F32)
                nc.sync.dma_start(out=whh_sb[:], in_=w_hhT[:])
                bst_sb = lp.tile([2, 4 * D], F32)
                nc.sync.dma_start(out=bst_sb[:], in_=bstack[:])
                one2_sb = lp.tile([2, D], F32)
                nc.sync.dma_start(out=one2_sb[:], in_=ones2[:])
                c_sb = lp.tile([D, D], F32)
                nc.vector.memset(c_sb[:], 0)

                for t in range(5):
                    pg = lps.tile([D, 4 * D], F32, space="PSUM", tag="pg")
                    nc.tensor.matmul(out=pg[:], lhsT=w_sb[:], rhs=wih_sb[:],
                                     start=True, stop=False)
                    if t > 0:
                        nc.tensor.matmul(out=pg[:], lhsT=w_sb[:], rhs=whh_sb[:],
                                         start=False, stop=False)
                    nc.tensor.matmul(out=pg[:], lhsT=one2_sb[:], rhs=bst_sb[:],
                                     start=False, stop=True)
                    sif = lp.tile([D, 2 * D], F32, tag="sif")
                    nc.scalar.activation(out=sif[:], in_=pg[:, 0:2 * D], func=AF.Sigmoid)
                    gt = lp.tile([D, D], F32, tag="gt")
                    nc.scalar.activation(out=gt[:], in_=pg[:, 2 * D:3 * D], func=AF.Tanh)
                    ot = lp.tile([D, D], F32, tag="ot")
                    nc.scalar.activation(out=ot[:], in_=pg[:, 3 * D:4 * D], func=AF.Sigmoid)
                    t1 = lp.tile([D, D], F32, tag="t1")
                    nc.vector.tensor_tensor(out=t1[:], in0=sif[:, 0:D], in1=gt[:], op=OP.mult)
                    nc.vector.tensor_tensor(out=c_sb[:], in0=sif[:, D:2 * D], in1=c_sb[:], op=OP.mult)
                    nc.vector.tensor_tensor(out=c_sb[:], in0=c_sb[:], in1=t1[:], op=OP.add)
                    th = lp.tile([D, D], F32, tag="th")
                    nc.scalar.activation(out=th[:], in_=c_sb[:], func=AF.Tanh)
                    hh = lp.tile([D, D], F32, tag="hh")
                    nc.vector.tensor_tensor(out=hh[:], in0=ot[:], in1=th[:], op=OP.mult)
                    pw = lps.tile([D, D], F32, space="PSUM", tag="pw")
                    nc.tensor.transpose(out=pw[:], in_=hh[:], identity=ident_sb[:D, :D])
                    nc.vector.tensor_copy(out=w_sb[:], in_=pw[:])
                nc.vector.tensor_copy(out=w5b[:], in_=w_sb[:])

            # ---- phase 2: gather x4d[src], one-hot scatter, W5 fold, relu ----
            with (
                tc.tile_pool(name="p2m", bufs=3) as p2m,
                tc.tile_pool(name="p2c", bufs=3) as p2c,
                tc.tile_pool(name="p2o", bufs=4) as p2o,
                tc.tile_pool(name="p2ps", bufs=4, space="PSUM") as p2ps,
            ):
                for gi, g in enumerate(cfg.groups):
                    glist = list(g)
                    # gathers per chunk, then bf16 cast on (idle) ScalarE --
                    # also frees the fp32 gather buffer quickly
                    mb = []
                    for qq in range(4):
                        J = int(capT[glist, qq].sum())
                        mb.append(None)
                        if J == 0:
                            continue
                        buf = p2m.tile([128, J, H], F32, tag=f"mb{qq}")
                        base = int(g_off[glist[0], qq])
                        gix = p2m.tile([128, J * 8], I16, tag=f"gix{qq}")
                        nc.sync.dma_start(out=gix[:],
                                          in_=gidx[:, base // 16:(base + J * 128) // 16])
                        nc.gpsimd.dma_gather(
                            out_ap=buf[:],
                            in_ap=x4d[qq * CS:(qq + 1) * CS, :],
                            idxs_ap=gix[:],
                            num_idxs=J * 128,
                            num_idxs_reg=J * 128,
                            elem_size=H,
                            queue_num=qq % NQ, single_packet=False,
                        )
                        bufb = p2c.tile([128, J, H], BF16, tag=f"mbb{qq}")
                        nc.scalar.activation(out=bufb[:], in_=buf[:], func=AF.Copy)
                        mb[qq] = bufb
                    # dl slab
                    Tg = int(capT[glist, :].sum())
                    dl_sb = p2o.tile([128, Tg], BF16, tag="dl")
                    tg0 = int(t_off[glist[0], 0])
                    nc.sync.dma_start(out=dl_sb[:], in_=dl_all[:, tg0:tg0 + Tg])
                    for wi, w in enumerate(glist):
                        Tw = int(capT[w, :].sum())
                        if Tw == 0:
                            continue
                        oh = p2o.tile([128, Tw, 128], BF16, tag="oh")
                        dloff = int(t_off[w, 0]) - tg0
                        nc.vector.tensor_tensor(
                            out=oh[:],
                            in0=iotab_sb[:, None, :].broadcast_to([128, Tw, 128]),
                            in1=dl_sb[:, dloff:dloff + Tw, None].broadcast_to([128, Tw, 128]),
                            op=OP.is_equal)
                        pz = p2ps.tile([H, 128], F32, space="PSUM", tag="pz")
                        n_mm = 0
                        for qq in range(4):
                            nt = int(capT[w, qq])
                            if nt == 0:
                                continue
                            mcol = int(g_off[w, qq] - g_off[glist[0], qq]) // 128
                            ohcol = int(t_off[w, qq]) - tg0 - dloff
                            for j in range(nt):
                                nc.tensor.matmul(
                                    out=pz[:],
                                    lhsT=mb[qq][:, mcol + j, :],
                                    rhs=oh[:, ohcol + j, :],
                                    start=(n_mm == 0), stop=(n_mm == Tw - 1))
                                n_mm += 1
                        # Sx -> SBUF (bf16), then xl^T = relu(W5^T @ Sx)
                        sxs = p2o.tile([H, 128], BF16, tag="sxs")
                        nc.vector.tensor_copy(out=sxs[:], in_=pz[:])
                        pxl = p2ps.tile([H, 128], F32, space="PSUM", tag="pxl")
                        nc.tensor.matmul(out=pxl[:], lhsT=w5b[:], rhs=sxs[:],
                                         start=True, stop=True)
                        nc.scalar.activation(out=xts[:, w * 128:(w + 1) * 128],
                                             in_=pxl[:], func=AF.Relu)

            # ---- phase 3: UV = dinv * (xl @ [W1a|W1b]), permuted rows, bf16 ----
            GB3 = 7
            with (
                tc.tile_pool(name="p3", bufs=3) as p3,
                tc.tile_pool(name="p3ps", bufs=4, space="PSUM") as p3ps,
            ):
                for b0 in range(0, WPC, GB3):
                    nb = min(GB3, WPC - b0)
                    uvw = p3.tile([128, GB3, 2 * H], BF16, tag="uvw")
                    for k in range(nb):
                        w = b0 + k
                        puv = p3ps.tile([128, 2 * H], F32, space="PSUM", tag="puv")
                        nc.tensor.matmul(out=puv[:],
                                         lhsT=xts[:, w * 128:(w + 1) * 128],
                                         rhs=W1ab_sb[:], start=True, stop=True)
                        nc.vector.tensor_scalar(
                            out=uvw[:, k, :], in0=puv[:],
                            scalar1=dinv_sh[:, w:w + 1], scalar2=None, op0=OP.mult)
                    # node (p, w) -> UV_shard row p*WPC + w  (contiguous per partition)
                    nc.sync.dma_start(
                        out=UV_shard.rearrange("(p W) f -> p W f", p=128)[:, b0:b0 + nb, :],
                        in_=uvw[:, :nb, :].bitcast(F32))

            # ---- preload phase-4 first-superslab index tiles (before the
            # collective: everything issued after it waits for it) ----
            SSL, SL = cfg.SSL, cfg.SL
            with (
                tc.tile_pool(name="p4i", bufs=1) as p4i,
                tc.tile_pool(name="p4g", bufs=4) as p4g,
                tc.tile_pool(name="p4e", bufs=3) as p4e,
                tc.tile_pool(name="p4h", bufs=2) as p4h,
                tc.tile_pool(name="p4ps", bufs=4, space="PSUM") as p4ps,
            ):
                pre_six, pre_dix = {}, {}
                for cell in range(16):
                    nt0 = min(SSL, int(capE[cell]))
                    if nt0 <= 0:
                        continue
                    base = int(cell_off[cell]) * 128
                    t6 = p4i.tile([128, SSL * 8], I16, tag=f"p6_{cell}")
                    nc.sync.dma_start(out=t6[:, :nt0 * 8],
                                      in_=sidx[:, base // 16:(base + nt0 * 128) // 16])
                    t7 = p4i.tile([128, SSL * 8], I16, tag=f"p7_{cell}")
                    nc.sync.dma_start(out=t7[:, :nt0 * 8],
                                      in_=didx[:, base // 16:(base + nt0 * 128) // 16])
                    pre_six[cell], pre_dix[cell] = t6, t7

                # ---- allgather UV ----
                nc.gpsimd.collective_compute(
                    "AllGather", OP.bypass,
                    replica_groups=[list(range(C))],
                    ins=[UV_shard[:]], outs=[UV_full[:]])

                # ---- phase 4: edge MLP ----
                lg_sb = pp.tile([128, T4], F32)
                qload = [0, 0, 0, 0]  # tiles assigned per queue (greedy balance)
                qcnt = [0, 0, 0, 0]   # preps issued per queue (sem bookkeeping)

                def pick_q(nt):
                    q = min(range(NQ), key=lambda i: qload[i])
                    qload[q] += nt
                    return q
                for cell in range(16):
                    a, b = cell // 4, cell % 4
                    ctiles = int(capE[cell])
                    coff = int(cell_off[cell])
                    for s0 in range(0, ctiles, SSL):
                        nt = min(SSL, ctiles - s0)
                        base = (coff + s0) * 128
                        if s0 == 0:
                            six, dix = pre_six[cell], pre_dix[cell]
                        else:
                            six = p4g.tile([128, SSL * 8], I16, tag="six")
                            nc.sync.dma_start(out=six[:, :nt * 8],
                                              in_=sidx[:, base // 16:(base + nt * 128) // 16])
                            dix = p4g.tile([128, SSL * 8], I16, tag="dix")
                            nc.sync.dma_start(out=dix[:, :nt * 8],
                                              in_=didx[:, base // 16:(base + nt * 128) // 16])
                        usb = p4g.tile([128, SSL, H], F32, tag="usb")
                        uq = pick_q(nt)
                        nc.gpsimd.dma_gather(
                            out_ap=usb[:, :nt, :],
                            in_ap=UV_full[a * CS:(a + 1) * CS, :],
                            idxs_ap=six[:, :nt * 8],
                            num_idxs=nt * 128, num_idxs_reg=nt * 128,
                            elem_size=H, queue_num=uq, single_packet=False,
                            prepare_only=True, sem=qsems[uq])
                        nc.gpsimd.trigger_dma(count=None, queue_num=uq)
                        qcnt[uq] += 1
                        uwait = 16 * qcnt[uq]
                        vsb = p4g.tile([128, SSL, H], F32, tag="vsb")
                        vq = pick_q(nt)
                        nc.gpsimd.dma_gather(
                            out_ap=vsb[:, :nt, :],
                            in_ap=UV_full[b * CS:(b + 1) * CS, :],
                            idxs_ap=dix[:, :nt * 8],
                            num_idxs=nt * 128, num_idxs_reg=nt * 128,
                            elem_size=H, queue_num=vq, single_packet=False,
                            prepare_only=True, sem=qsems[vq])
                        nc.gpsimd.trigger_dma(count=None, queue_num=vq)
                        qcnt[vq] += 1
                        vwait = 16 * qcnt[vq]
                        easb = p4e.tile([FE + 1, SSL * 128], BF16, tag="easb")
                        nc.sync.dma_start(out=easb[:, :nt * 128],
                                          in_=ea_t[:, base:base + nt * 128])
                        # data-completion waits (descriptor-baked sems), then
                        # U[src] + V[dst] presum on DVE (bf16 views of f32 rows)
                        nc.vector.wait_ge(qsems[uq], uwait)
                        nc.vector.wait_ge(qsems[vq], vwait)
                        uvsum = p4e.tile([128, SSL, H], BF16, tag="uvsum")
                        nc.vector.tensor_tensor(
                            out=uvsum[:, :nt, :],
                            in0=usb[:, :nt, 0:H // 2].bitcast(BF16),
                            in1=vsb[:, :nt, H // 2:H].bitcast(BF16), op=OP.add)
                        for t0 in range(0, nt, SL):
                            nsl = min(SL, nt - t0)
                            ph = p4ps.tile([128, SL * H], F32, space="PSUM", tag="ph")
                            nc.tensor.matmul(
                                out=ph[:, :nsl * H],
                                lhsT=identb_sb[:],
                                rhs=uvsum[:, t0:t0 + nsl, :].rearrange("p t h -> p (t h)"),
                                start=True, stop=False)
                            for t in range(nsl):
                                nc.tensor.matmul(
                                    out=ph[:, t * H:(t + 1) * H],
                                    lhsT=easb[:, (t0 + t) * 128:(t0 + t + 1) * 128],
                                    rhs=W1cb_sb[:],
                                    start=False, stop=(t == nsl - 1))
                            hid = p4h.tile([128, SL, H], BF16, tag="hid")
                            nc.scalar.activation(
                                out=hid[:, :nsl, :],
                                in_=ph[:, :nsl * H].rearrange("p (t h) -> p t h", t=nsl),
                                func=AF.Relu)
                            prod = p4h.tile([128, SL, H], F32, tag="prod")
                            nc.vector.tensor_tensor(
                                out=prod[:, :nsl, :], in0=hid[:, :nsl, :],
                                in1=w2b_sb[:, None, :].broadcast_to([128, nsl, H]),
                                op=OP.mult)
                            nc.vector.tensor_reduce(
                                out=lg_sb[:, coff + s0 + t0:coff + s0 + t0 + nsl],
                                in_=prod[:, :nsl, :],
                                axis=mybir.AxisListType.X, op=OP.add)
                # + b2, write out
                nc.vector.tensor_scalar(
                    out=lg_sb[:], in0=lg_sb[:], scalar1=b2b_sb[:, 0:1], scalar2=None,
                    op0=OP.add)
                nc.sync.dma_start(out=logits_out[:], in_=lg_sb[:])

    nc.compile()
    return nc


# ---------------- numpy reference (mirrors the jax reference) ----------------

def numpy_ref(inputs):
    xs = np.asarray(inputs["xs"], np.float32)
    ei = np.asarray(inputs["edge_index"])
    ea = np.asarray(inputs["edge_attr"], np.float32)
    N = xs.shape[1]
    src = ei[0].astype(np.int64)
    dst = ei[1].astype(np.int64)
    loops = np.arange(N)
    srcA = np.concatenate([src, loops])
    dstA = np.concatenate([dst, loops])
    deg = np.bincount(dstA, minlength=N).astype(np.float32)
    dinv = np.where(deg > 0, deg ** -0.5, 0.0).astype(np.float32)

    def sig(x):
        return (1.0 / (1.0 + np.exp(-x))).astype(np.float32)

    W = np.asarray(inputs["init_w"], np.float32)
    w_ih = np.asarray(inputs["w_ih"], np.float32)
    w_hh = np.asarray(inputs["w_hh"], np.float32)
    b = (np.asarray(inputs["b_ih"], np.float32) + np.asarray(inputs["b_hh"], np.float32))
    h = np.zeros((64, 64), np.float32)
    c = np.zeros((64, 64), np.float32)
    for t in range(xs.shape[0]):
        gates = W.T @ w_ih.T + h @ w_hh.T + b
        i, f, g, o = np.split(gates, 4, axis=1)
        c = sig(f) * c + sig(i) * np.tanh(g)
        h = sig(o) * np.tanh(c)
        W = h.T.copy()

    y = dinv[:, None] * (xs[-1] @ W)
    S = np.zeros((N, 64), np.float32)
    np.add.at(S, dstA, y[srcA])
    xl = np.maximum(S, 0.0)
    mlp_w1 = np.asarray(inputs["mlp_w1"], np.float32)
    U = dinv[:, None] * (xl @ mlp_w1[:64])
    V = dinv[:, None] * (xl @ mlp_w1[64:128])
    Cc = ea @ mlp_w1[128:] + np.asarray(inputs["mlp_b1"], np.float32)
    hid = np.maximum(U[src] + V[dst] + Cc, 0.0)
    return (hid @ np.asarray(inputs["mlp_w2"], np.float32))[:, 0] + np.asarray(inputs["mlp_b2"], np.float32)[0]


# ------------------------------ kernel entry ------------------------------

_CACHE = {}


def kernel(**inputs):
    """Full-input EvolvingGNN kernel on 8 TRN2 NeuronCores."""
    N = int(inputs["xs"].shape[1])
    E = int(inputs["edge_index"].shape[1])
    cfg = Cfg(N, E)
    in_maps, static, meta = prep(inputs, cfg)
    key = (N, E, tuple(static["capT"].ravel()), tuple(static["capE"].ravel()))
    nc = _CACHE.get(key)
    if nc is None:
        nc = build(cfg, static)
        _CACHE[key] = nc
    r = run_bass_kernel_spmd(nc, in_maps, core_ids=list(range(C)))
    return unshard(r.results, meta, E)


# revision 40
# speedup vs baseline: 2.1817x; 2.1817x over previous
"""EvolvingGNN (gnn_message_passing) kernel for 8 Trainium2 NeuronCores.

Self-contained: takes the full inputs of setup_inputs(), returns the full
[E] float32 logits.  See kernel() at the bottom.

Structure:
  host: dinv-scaled x4d (so messages gather straight from the input and the
        device-side phase-1 y materialization disappears -- segment_sum
        commutes with the @W5 right-matmul, applied per window after
        aggregation); balanced dst relabeling node -> (core, window,
        partition) equalizes per-(core,window,chunk) message counts across
        cores (gather capacities are max-over-cores); edge bucketing by
        (dst-core, dst-window, src-chunk) for P2 and (src-chunk, dst-chunk)
        cells for P4, with UV rows in a partition-major permuted layout so
        the UV_shard DMA writes are per-partition contiguous.
  P0: LSTM -> W5 (tiny).
  P2: gather x4d[src] (dma_gather spread over all 4 SWDGE queues -- the
      per-queue descriptor pacing is the bottleneck resource), cast bf16 on
      ScalarE, one-hot (iota==dl) scatter-add via TensorE into PSUM, then
      xl^T = relu(W5^T @ Sx) per window, kept in SBUF.
  P3: per window U||V = dinv * (xl @ [W1a|W1b]) bf16 packed in 256B rows,
      single AllGather bitcast to f32 (halves collective element count).
  P4: per edge: gather UV[src], UV[dst] (256B rows, greedy queue balance),
      U+V presummed on DVE, + ea@W1cb in PSUM, hid = relu(.),
      logits = hid @ w2 + b2.  First-superslab index tiles are preloaded
      before the collective (everything issued after it waits on it).
"""


import heapq

import numpy as np
import ml_dtypes

import concourse.bacc as bacc
import concourse.tile as tile
from concourse import mybir
from concourse.bass_utils import run_bass_kernel_spmd

F32 = mybir.dt.float32
BF16 = mybir.dt.bfloat16
I16 = mybir.dt.int16
AF = mybir.ActivationFunctionType
OP = mybir.AluOpType
BF_NP = np.dtype(ml_dtypes.bfloat16)

D = 64   # input/lstm dim
H = 64   # hidden dim
FE = 16  # edge feature dim
C = 8    # cores
NQ = 4   # SWDGE queues


class Cfg:
    def __init__(self, N, E, gw=6, ssl=32, sl=8):
        self.N, self.E = N, E
        self.NP = -(-N // (C * 128)) * C * 128      # padded nodes
        self.SHARD = self.NP // C                    # nodes per core
        self.WPC = self.SHARD // 128                 # windows per core
        self.CS = self.NP // 4                       # gather chunk rows
        assert self.CS < 32768, "chunk exceeds int16 gather range"
        assert E % C == 0
        self.EPC = E // C                            # edges per core
        self.GW = gw                                 # windows per ph2 gather group
        self.SSL = ssl                               # ph4 superslab tiles
        self.SL = sl                                 # ph4 inner slab tiles
        self.groups = [range(s, min(s + gw, self.WPC))
                       for s in range(0, self.WPC, gw)]


def _wrap16(arr_i16):
    """[S] -> [128, S/16] wrapped in 16 partitions, replicated x8."""
    w = arr_i16.reshape(-1, 16).T  # [16, S/16]
    return np.ascontiguousarray(np.tile(w, (8, 1)))


def _ranks_after_sort(sorted_keys):
    """rank of each element within its (contiguous) key run."""
    n = len(sorted_keys)
    if n == 0:
        return np.zeros(0, np.int64)
    change = np.r_[True, sorted_keys[1:] != sorted_keys[:-1]]
    starts = np.where(change)[0]
    return np.arange(n) - np.repeat(starts, np.diff(np.r_[starts, n]))


def prep(inputs, cfg):
    """Host preprocessing. Returns (in_maps, static, meta)."""
    N, E, NP, CS = cfg.N, cfg.E, cfg.NP, cfg.CS
    WPC, SHARD = cfg.WPC, cfg.SHARD

    ei = np.asarray(inputs["edge_index"])
    src = ei[0].astype(np.int64)
    dst = ei[1].astype(np.int64)
    loops = np.arange(N, dtype=np.int64)
    srcA = np.concatenate([src, loops])
    dstA = np.concatenate([dst, loops])

    deg = np.bincount(dstA, minlength=NP).astype(np.float32)
    deg[deg == 0] = 1.0
    dinv = (deg ** -0.5).astype(np.float32)

    # dinv-scaled input features (message payload), padded, ORIGINAL node order
    xs = np.asarray(inputs["xs"], dtype=np.float32)
    x4d = np.zeros((NP, D), np.float32)
    x4d[:N] = dinv[:N, None] * xs[-1]

    q = srcA // CS

    # ---- balanced dst relabeling: node -> (core, window, partition) so that
    # per-(core,window,chunk) message counts are even across cores (the
    # gather capacities are max-over-cores; balance cuts padded slots ~13%)
    tot = np.bincount(dstA, minlength=NP)
    order_n = np.argsort(-tot, kind="stable")
    NB = C * WPC
    heap = [(0, b) for b in range(NB)]
    heapq.heapify(heap)
    bucket_of = np.zeros(NP, np.int64)
    p_of = np.zeros(NP, np.int64)
    fill = np.zeros(NB, np.int64)
    for n in order_n:
        while True:
            load, b = heapq.heappop(heap)
            if fill[b] < 128:
                break
        bucket_of[n] = b
        p_of[n] = fill[b]
        fill[b] += 1
        if fill[b] < 128:
            heapq.heappush(heap, (load + int(tot[n]), b))
    core_of = bucket_of // WPC
    win_of = bucket_of % WPC
    # UV row of node n (partition-major within its shard)
    uvrow_of = core_of * SHARD + p_of * WPC + win_of
    dinv_sh_arr = np.zeros(NB * 128, np.float32)
    dinv_sh_arr[bucket_of * 128 + p_of] = dinv
    dinv_shard = [np.ascontiguousarray(
        dinv_sh_arr.reshape(C, WPC, 128)[c].T) for c in range(C)]

    # ---- phase 2 bucketing: key = (core, window, chunk) ----
    core = core_of[dstA]
    win = win_of[dstA]
    dloc = p_of[dstA].astype(np.float32)
    sloc = (srcA % CS).astype(np.int16)

    key = (core * WPC + win) * 4 + q
    order = np.argsort(key, kind="stable")
    skey = key[order]
    counts = np.bincount(key, minlength=C * WPC * 4).reshape(C, WPC, 4)
    capT = np.ceil(counts.max(axis=0) / 128).astype(np.int64)      # [WPC, 4] tiles

    # gather-stream order: (group, chunk, window) ; dl-stream order: (window, chunk)
    slots_wq = capT * 128
    g_off = np.zeros((WPC, 4), np.int64)     # slot offset in gather stream
    t_off = np.zeros((WPC, 4), np.int64)     # tile offset in dl stream
    pos = 0
    for g in cfg.groups:
        for qq in range(4):
            for w in g:
                g_off[w, qq] = pos
                pos += slots_wq[w, qq]
    total_slots2 = pos
    pos = 0
    for w in range(WPC):
        for qq in range(4):
            t_off[w, qq] = pos
            pos += capT[w, qq]
    total_tiles2 = pos

    ranks = _ranks_after_sort(skey)
    gpos = g_off[win[order], q[order]] + ranks                     # slot in gather stream
    dpos = t_off[win[order], q[order]] * 128 + ranks               # slot in dl stream
    corder = core[order]

    gidx_list, dl_list = [], []
    for c in range(C):
        m = corder == c
        garr = np.zeros(total_slots2, np.int16)
        garr[gpos[m]] = sloc[order][m]
        dlarr = np.full(total_tiles2 * 128, -1.0, np.float32)
        dlarr[dpos[m]] = dloc[order][m]
        gidx_list.append(_wrap16(garr))
        dl_list.append(np.ascontiguousarray(dlarr.reshape(-1, 128).T.astype(BF_NP)))

    # ---- phase 4 bucketing: per core (positional edge shard), key = (a, b) ----
    # UV rows live in the relabeled partition-major layout (uvrow_of)
    EPC = cfg.EPC
    perm_row = uvrow_of[src]
    perm_col = uvrow_of[dst]
    a_all = perm_row // CS
    b_all = perm_col // CS
    sl4 = (perm_row % CS).astype(np.int16)
    dl4 = (perm_col % CS).astype(np.int16)

    counts4 = np.zeros((C, 16), np.int64)
    key4 = (a_all * 4 + b_all)
    for c in range(C):
        counts4[c] = np.bincount(key4[c * EPC:(c + 1) * EPC], minlength=16)
    capE = np.ceil(counts4.max(axis=0) / 128).astype(np.int64)     # [16] tiles
    cell_off = np.r_[0, np.cumsum(capE)][:16]                       # tile offsets
    total_tiles4 = int(capE.sum())
    total_slots4 = total_tiles4 * 128

    ea = np.asarray(inputs["edge_attr"], dtype=np.float32)
    sidx_list, didx_list, eat_list, origmap = [], [], [], []
    for c in range(C):
        sl_ = slice(c * EPC, (c + 1) * EPC)
        k = key4[sl_]
        o = np.argsort(k, kind="stable")
        r = _ranks_after_sort(k[o])
        slotpos = cell_off[k[o]] * 128 + r
        sarr = np.zeros(total_slots4, np.int16)
        darr = np.zeros(total_slots4, np.int16)
        eat = np.zeros((17, total_slots4), np.float32)
        om = np.full(total_slots4, -1, np.int64)
        eids = np.arange(c * EPC, (c + 1) * EPC)[o]
        sarr[slotpos] = sl4[sl_][o]
        darr[slotpos] = dl4[sl_][o]
        eat[:16, slotpos] = ea[eids].T
        eat[16, slotpos] = 1.0
        om[slotpos] = eids
        sidx_list.append(_wrap16(sarr))
        didx_list.append(_wrap16(darr))
        eat_list.append(np.ascontiguousarray(eat.astype(BF_NP)))
        origmap.append(om)

    # ---- dense weights ----
    w_ihT = np.ascontiguousarray(np.asarray(inputs["w_ih"], np.float32).T)   # [64, 256]
    w_hhT = np.ascontiguousarray(np.asarray(inputs["w_hh"], np.float32).T)
    bstack = np.stack([np.asarray(inputs["b_ih"], np.float32),
                       np.asarray(inputs["b_hh"], np.float32)])              # [2, 256]
    ones2 = np.ones((2, D), np.float32)
    init_w = np.asarray(inputs["init_w"], np.float32)
    mlp_w1 = np.asarray(inputs["mlp_w1"], np.float32)
    W1ab = np.ascontiguousarray(
        np.hstack([mlp_w1[:H], mlp_w1[H:2 * H]]).astype(BF_NP))              # [64, 128]
    W1cb = np.ascontiguousarray(
        np.vstack([mlp_w1[2 * H:],
                   np.asarray(inputs["mlp_b1"], np.float32)[None]]).astype(BF_NP))  # [17, 64]
    w2b = np.ascontiguousarray(
        np.tile(np.asarray(inputs["mlp_w2"], np.float32).T, (128, 1)).astype(BF_NP))  # [128, 64]
    b2b = np.full((128, 1), np.asarray(inputs["mlp_b2"], np.float32)[0], np.float32)

    in_maps = []
    for c in range(C):
        in_maps.append(dict(
            x4d=x4d, dinvS=dinv_shard[c],
            gidx=gidx_list[c], dl_all=dl_list[c],
            sidx=sidx_list[c], didx=didx_list[c], ea_t=eat_list[c],
            w_ihT=w_ihT, w_hhT=w_hhT, bstack=bstack, ones2=ones2,
            init_w=init_w, W1ab=W1ab, W1cb=W1cb, w2b=w2b, b2b=b2b,
        ))

    static = dict(capT=capT, g_off=g_off, t_off=t_off,
                  total_slots2=total_slots2, total_tiles2=total_tiles2,
                  capE=capE, cell_off=cell_off,
                  total_slots4=total_slots4, total_tiles4=total_tiles4)
    meta = dict(origmap=origmap, total_tiles4=total_tiles4)
    return in_maps, static, meta


def unshard(results, meta, E):
    logits = np.zeros(E, np.float32)
    for c in range(C):
        out = np.asarray(results[c]["logits_out"])  # [128, T4]
        flat = out.T.reshape(-1)                    # slot j*128+p at [p, j] -> flat[j*128+p]
        om = meta["origmap"][c]
        m = om >= 0
        logits[om[m]] = flat[m]
    return logits


def build(cfg, static):
    NP, CS, WPC, SHARD = cfg.NP, cfg.CS, cfg.WPC, cfg.SHARD
    capT, g_off, t_off = static["capT"], static["g_off"], static["t_off"]
    S2, T2 = static["total_slots2"], static["total_tiles2"]
    capE, cell_off = static["capE"], static["cell_off"]
    S4, T4 = static["total_slots4"], static["total_tiles4"]

    nc = bacc.Bacc("TRN2", target_bir_lowering=False, num_devices=C,
                   num_swdge_queues=NQ)

    # ---- params ----
    P = lambda name, shape, dt=F32: nc.declare_dram_parameter(name, list(shape), dt, isOutput=False)
    x4d = P("x4d", [NP, D])
    dinvS = P("dinvS", [128, WPC])
    gidx = P("gidx", [128, S2 // 16], I16)
    dl_all = P("dl_all", [128, T2], BF16)
    sidx = P("sidx", [128, S4 // 16], I16)
    didx = P("didx", [128, S4 // 16], I16)
    ea_t = P("ea_t", [17, S4], BF16)
    w_ihT = P("w_ihT", [D, 4 * D])
    w_hhT = P("w_hhT", [D, 4 * D])
    bstack = P("bstack", [2, 4 * D])
    ones2 = P("ones2", [2, D])
    init_w = P("init_w", [D, H])
    W1ab = P("W1ab", [H, 2 * H], BF16)
    W1cb = P("W1cb", [FE + 1, H], BF16)
    w2b = P("w2b", [128, H], BF16)
    b2b = P("b2b", [128, 1])
    logits_out = nc.declare_dram_parameter("logits_out", [128, T4], F32, isOutput=True)

    # ---- internal dram ----
    # UV is bf16 [*, 2H] data stored/collected as f32 [*, H] (pure bitcast):
    # halves the AllGather element count and keeps 256B gather elements.
    UV_shard = nc.dram_tensor("UV_shard", [SHARD, H], F32)
    UV_full = nc.dram_tensor("UV_full", [NP, H], F32, addr_space="Shared")

    iota_np = np.tile(np.arange(128, dtype=np.float32), (128, 1))
    iota_dram = nc.inline_tensor(iota_np, name="iota128")
    ident_dram = nc.inline_tensor(np.eye(128, dtype=np.float32), name="ident128")

    with tile.TileContext(nc) as tc:
        with tc.tile_pool(name="persist", bufs=1) as pp:
            iota_sb = pp.tile([128, 128], F32)
            nc.sync.dma_start(out=iota_sb[:], in_=iota_dram[:])
            iotab_sb = pp.tile([128, 128], BF16)
            nc.vector.tensor_copy(out=iotab_sb[:], in_=iota_sb[:])
            ident_sb = pp.tile([128, 128], F32)
            nc.sync.dma_start(out=ident_sb[:], in_=ident_dram[:])
            identb_sb = pp.tile([128, 128], BF16)
            nc.vector.tensor_copy(out=identb_sb[:], in_=ident_sb[:])
            w2b_sb = pp.tile([128, H], BF16)
            nc.sync.dma_start(out=w2b_sb[:], in_=w2b[:])
            b2b_sb = pp.tile([128, 1], F32)
            nc.sync.dma_start(out=b2b_sb[:], in_=b2b[:])
            W1ab_sb = pp.tile([H, 2 * H], BF16)
            nc.sync.dma_start(out=W1ab_sb[:], in_=W1ab[:])
            W1cb_sb = pp.tile([FE + 1, H], BF16)
            nc.sync.dma_start(out=W1cb_sb[:], in_=W1cb[:])
            dinv_sh = pp.tile([128, WPC], F32)
            nc.sync.dma_start(out=dinv_sh[:], in_=dinvS[:])
            # xl^T of this core's shard, bf16, lives in SBUF for all of P2/P3
            xts = pp.tile([H, SHARD], BF16)

            # ---- phase 0: LSTM -> W5 in w_sb [D, H]; bf16 copy in w5b ----
            w_sb = pp.tile([D, H], F32)
            w5b = pp.tile([D, H], BF16)
            nc.sync.dma_start(out=w_sb[:], in_=init_w[:])
            with (
                tc.tile_pool(name="lstm", bufs=1) as lp,
                tc.tile_pool(name="lstm_ps", bufs=2, space="PSUM") as lps,
            ):
                wih_sb = lp.tile([D, 4 * D], F32)
                nc.sync.dma_start(out=wih_sb[:], in_=w_ihT[:])
                whh_sb = lp.tile([D, 4 * D], F32)
                nc.sync.dma_start(out=whh_sb[:], in_=w_hhT[:])
                bst_sb = lp.tile([2, 4 * D], F32)
                nc.sync.dma_start(out=bst_sb[:], in_=bstack[:])
                one2_sb = lp.tile([2, D], F32)
                nc.sync.dma_start(out=one2_sb[:], in_=ones2[:])
                c_sb = lp.tile([D, D], F32)
                nc.vector.memset(c_sb[:], 0)

                for t in range(5):
                    pg = lps.tile([D, 4 * D], F32, space="PSUM", tag="pg")
                    nc.tensor.matmul(out=pg[:], lhsT=w_sb[:], rhs=wih_sb[:],
                                     start=True, stop=False)
                    if t > 0:
                        nc.tensor.matmul(out=pg[:], lhsT=w_sb[:], rhs=whh_sb[:],
                                         start=False, stop=False)
                    nc.tensor.matmul(out=pg[:], lhsT=one2_sb[:], rhs=bst_sb[:],
                                     start=False, stop=True)
                    sif = lp.tile([D, 2 * D], F32, tag="sif")
                    nc.scalar.activation(out=sif[:], in_=pg[:, 0:2 * D], func=AF.Sigmoid)
                    gt = lp.tile([D, D], F32, tag="gt")
                    nc.scalar.activation(out=gt[:], in_=pg[:, 2 * D:3 * D], func=AF.Tanh)
                    ot = lp.tile([D, D], F32, tag="ot")
                    nc.scalar.activation(out=ot[:], in_=pg[:, 3 * D:4 * D], func=AF.Sigmoid)
                    t1 = lp.tile([D, D], F32, tag="t1")
                    nc.vector.tensor_tensor(out=t1[:], in0=sif[:, 0:D], in1=gt[:], op=OP.mult)
                    nc.vector.tensor_tensor(out=c_sb[:], in0=sif[:, D:2 * D], in1=c_sb[:], op=OP.mult)
                    nc.vector.tensor_tensor(out=c_sb[:], in0=c_sb[:], in1=t1[:], op=OP.add)
                    th = lp.tile([D, D], F32, tag="th")
                    nc.scalar.activation(out=th[:], in_=c_sb[:], func=AF.Tanh)
                    hh = lp.tile([D, D], F32, tag="hh")
                    nc.vector.tensor_tensor(out=hh[:], in0=ot[:], in1=th[:], op=OP.mult)
                    pw = lps.tile([D, D], F32, space="PSUM", tag="pw")
                    nc.tensor.transpose(out=pw[:], in_=hh[:], identity=ident_sb[:D, :D])
                    nc.vector.tensor_copy(out=w_sb[:], in_=pw[:])
                nc.vector.tensor_copy(out=w5b[:], in_=w_sb[:])

            # ---- phase 2: gather x4d[src], one-hot scatter, W5 fold, relu ----
            with (
                tc.tile_pool(name="p2m", bufs=3) as p2m,
                tc.tile_pool(name="p2c", bufs=3) as p2c,
                tc.tile_pool(name="p2o", bufs=4) as p2o,
                tc.tile_pool(name="p2ps", bufs=4, space="PSUM") as p2ps,
            ):
                for gi, g in enumerate(cfg.groups):
                    glist = list(g)
                    # gathers per chunk, then bf16 cast on (idle) ScalarE --
                    # also frees the fp32 gather buffer quickly
                    mb = []
                    for qq in range(4):
                        J = int(capT[glist, qq].sum())
                        mb.append(None)
                        if J == 0:
                            continue
                        buf = p2m.tile([128, J, H], F32, tag=f"mb{qq}")
                        base = int(g_off[glist[0], qq])
                        gix = p2m.tile([128, J * 8], I16, tag=f"gix{qq}")
                        nc.sync.dma_start(out=gix[:],
                                          in_=gidx[:, base // 16:(base + J * 128) // 16])
                        nc.gpsimd.dma_gather(
                            out_ap=buf[:],
                            in_ap=x4d[qq * CS:(qq + 1) * CS, :],
                            idxs_ap=gix[:],
                            num_idxs=J * 128,
                            num_idxs_reg=J * 128,
                            elem_size=H,
                            queue_num=qq % NQ, single_packet=False,
                        )
                        bufb = p2c.tile([128, J, H], BF16, tag=f"mbb{qq}")
                        nc.scalar.activation(out=bufb[:], in_=buf[:], func=AF.Copy)
                        mb[qq] = bufb
                    # dl slab
                    Tg = int(capT[glist, :].sum())
                    dl_sb = p2o.tile([128, Tg], BF16, tag="dl")
                    tg0 = int(t_off[glist[0], 0])
                    nc.sync.dma_start(out=dl_sb[:], in_=dl_all[:, tg0:tg0 + Tg])
                    for wi, w in enumerate(glist):
                        Tw = int(capT[w, :].sum())
                        if Tw == 0:
                            continue
                        oh = p2o.tile([128, Tw, 128], BF16, tag="oh")
                        dloff = int(t_off[w, 0]) - tg0
                        nc.vector.tensor_tensor(
                            out=oh[:],
                            in0=iotab_sb[:, None, :].broadcast_to([128, Tw, 128]),
                            in1=dl_sb[:, dloff:dloff + Tw, None].broadcast_to([128, Tw, 128]),
                            op=OP.is_equal)
                        pz = p2ps.tile([H, 128], F32, space="PSUM", tag="pz")
                        n_mm = 0
                        for qq in range(4):
                            nt = int(capT[w, qq])
                            if nt == 0:
                                continue
                            mcol = int(g_off[w, qq] - g_off[glist[0], qq]) // 128
                            ohcol = int(t_off[w, qq]) - tg0 - dloff
                            for j in range(nt):
                                nc.tensor.matmul(
                                    out=pz[:],
                                    lhsT=mb[qq][:, mcol + j, :],
                                    rhs=oh[:, ohcol + j, :],
                                    start=(n_mm == 0), stop=(n_mm == Tw - 1))
                                n_mm += 1
                        # Sx -> SBUF (bf16), then xl^T = relu(W5^T @ Sx)
                        sxs = p2o.tile([H, 128], BF16, tag="sxs")
                        nc.vector.tensor_copy(out=sxs[:], in_=pz[:])
                        pxl = p2ps.tile([H, 128], F32, space="PSUM", tag="pxl")
                        nc.tensor.matmul(out=pxl[:], lhsT=w5b[:], rhs=sxs[:],
                                         start=True, stop=True)
                        nc.scalar.activation(out=xts[:, w * 128:(w + 1) * 128],
                                             in_=pxl[:], func=AF.Relu)

            # ---- phase 3: UV = dinv * (xl @ [W1a|W1b]), permuted rows, bf16 ----
            GB3 = 7
            with (
                tc.tile_pool(name="p3", bufs=3) as p3,
                tc.tile_pool(name="p3ps", bufs=4, space="PSUM") as p3ps,
            ):
                for b0 in range(0, WPC, GB3):
                    nb = min(GB3, WPC - b0)
                    uvw = p3.tile([128, GB3, 2 * H], BF16, tag="uvw")
                    for k in range(nb):
                        w = b0 + k
                        puv = p3ps.tile([128, 2 * H], F32, space="PSUM", tag="puv")
                        nc.tensor.matmul(out=puv[:],
                                         lhsT=xts[:, w * 128:(w + 1) * 128],
                                         rhs=W1ab_sb[:], start=True, stop=True)
                        nc.vector.tensor_scalar(
                            out=uvw[:, k, :], in0=puv[:],
                            scalar1=dinv_sh[:, w:w + 1], scalar2=None, op0=OP.mult)
                    # node (p, w) -> UV_shard row p*WPC + w  (contiguous per partition)
                    nc.sync.dma_start(
                        out=UV_shard.rearrange("(p W) f -> p W f", p=128)[:, b0:b0 + nb, :],
                        in_=uvw[:, :nb, :].bitcast(F32))

            # ---- preload phase-4 first-superslab index tiles (before the
            # collective: everything issued after it waits for it) ----
            SSL, SL = cfg.SSL, cfg.SL
            with (
                tc.tile_pool(name="p4i", bufs=1) as p4i,
                tc.tile_pool(name="p4g", bufs=4) as p4g,
                tc.tile_pool(name="p4e", bufs=3) as p4e,
                tc.tile_pool(name="p4h", bufs=2) as p4h,
                tc.tile_pool(name="p4ps", bufs=4, space="PSUM") as p4ps,
            ):
                pre_six, pre_dix = {}, {}
                for cell in range(16):
                    for sb0 in (0, SSL):
                        if sb0 >= int(capE[cell]):
                            continue
                        nt0 = min(SSL, int(capE[cell]) - sb0)
                        base = (int(cell_off[cell]) + sb0) * 128
                        t6 = p4i.tile([128, SSL * 8], I16, tag=f"p6_{cell}_{sb0}")
                        nc.sync.dma_start(out=t6[:, :nt0 * 8],
                                          in_=sidx[:, base // 16:(base + nt0 * 128) // 16])
                        t7 = p4i.tile([128, SSL * 8], I16, tag=f"p7_{cell}_{sb0}")
                        nc.sync.dma_start(out=t7[:, :nt0 * 8],
                                          in_=didx[:, base // 16:(base + nt0 * 128) // 16])
                        pre_six[cell, sb0], pre_dix[cell, sb0] = t6, t7

                # ---- allgather UV ----
                nc.gpsimd.collective_compute(
                    "AllGather", OP.bypass,
                    replica_groups=[list(range(C))],
                    ins=[UV_shard[:]], outs=[UV_full[:]])

                # ---- phase 4: edge MLP ----
                lg_sb = pp.tile([128, T4], F32)
                qload = [0, 0, 0, 0]  # tiles assigned per queue (greedy balance)

                def pick_q(nt):
                    q = min(range(NQ), key=lambda i: qload[i])
                    qload[q] += nt
                    return q
                for cell in range(16):
                    a, b = cell // 4, cell % 4
                    ctiles = int(capE[cell])
                    coff = int(cell_off[cell])
                    for s0 in range(0, ctiles, SSL):
                        nt = min(SSL, ctiles - s0)
                        base = (coff + s0) * 128
                        if (cell, s0) in pre_six:
                            six, dix = pre_six[cell, s0], pre_dix[cell, s0]
                        else:
                            six = p4g.tile([128, SSL * 8], I16, tag="six")
                            nc.sync.dma_start(out=six[:, :nt * 8],
                                              in_=sidx[:, base // 16:(base + nt * 128) // 16])
                            dix = p4g.tile([128, SSL * 8], I16, tag="dix")
                            nc.sync.dma_start(out=dix[:, :nt * 8],
                                              in_=didx[:, base // 16:(base + nt * 128) // 16])
                        usb = p4g.tile([128, SSL, H], F32, tag="usb")
                        nc.gpsimd.dma_gather(
                            out_ap=usb[:, :nt, :],
                            in_ap=UV_full[a * CS:(a + 1) * CS, :],
                            idxs_ap=six[:, :nt * 8],
                            num_idxs=nt * 128, num_idxs_reg=nt * 128,
                            elem_size=H, queue_num=pick_q(nt), single_packet=False)
                        vsb = p4g.tile([128, SSL, H], F32, tag="vsb")
                        nc.gpsimd.dma_gather(
                            out_ap=vsb[:, :nt, :],
                            in_ap=UV_full[b * CS:(b + 1) * CS, :],
                            idxs_ap=dix[:, :nt * 8],
                            num_idxs=nt * 128, num_idxs_reg=nt * 128,
                            elem_size=H, queue_num=pick_q(nt), single_packet=False)
                        easb = p4e.tile([FE + 1, SSL * 128], BF16, tag="easb")
                        nc.sync.dma_start(out=easb[:, :nt * 128],
                                          in_=ea_t[:, base:base + nt * 128])
                        # U[src] + V[dst] presum on DVE (bf16 views of f32 rows)
                        uvsum = p4e.tile([128, SSL, H], BF16, tag="uvsum")
                        nc.vector.tensor_tensor(
                            out=uvsum[:, :nt, :],
                            in0=usb[:, :nt, 0:H // 2].bitcast(BF16),
                            in1=vsb[:, :nt, H // 2:H].bitcast(BF16), op=OP.add)
                        for t0 in range(0, nt, SL):
                            nsl = min(SL, nt - t0)
                            ph = p4ps.tile([128, SL * H], F32, space="PSUM", tag="ph")
                            nc.tensor.matmul(
                                out=ph[:, :nsl * H],
                                lhsT=identb_sb[:],
                                rhs=uvsum[:, t0:t0 + nsl, :].rearrange("p t h -> p (t h)"),
                                start=True, stop=False)
                            for t in range(nsl):
                                nc.tensor.matmul(
                                    out=ph[:, t * H:(t + 1) * H],
                                    lhsT=easb[:, (t0 + t) * 128:(t0 + t + 1) * 128],
                                    rhs=W1cb_sb[:],
                                    start=False, stop=(t == nsl - 1))
                            hid = p4h.tile([128, SL, H], BF16, tag="hid")
                            nc.scalar.activation(
                                out=hid[:, :nsl, :],
                                in_=ph[:, :nsl * H].rearrange("p (t h) -> p t h", t=nsl),
                                func=AF.Relu)
                            prod = p4h.tile([128, SL, H], F32, tag="prod")
                            nc.vector.tensor_tensor(
                                out=prod[:, :nsl, :], in0=hid[:, :nsl, :],
                                in1=w2b_sb[:, None, :].broadcast_to([128, nsl, H]),
                                op=OP.mult)
                            nc.vector.tensor_reduce(
                                out=lg_sb[:, coff + s0 + t0:coff + s0 + t0 + nsl],
                                in_=prod[:, :nsl, :],
                                axis=mybir.AxisListType.X, op=OP.add)
                # + b2, write out
                nc.vector.tensor_scalar(
                    out=lg_sb[:], in0=lg_sb[:], scalar1=b2b_sb[:, 0:1], scalar2=None,
                    op0=OP.add)
                nc.sync.dma_start(out=logits_out[:], in_=lg_sb[:])

    nc.compile()
    return nc


# ---------------- numpy reference (mirrors the jax reference) ----------------

def numpy_ref(inputs):
    xs = np.asarray(inputs["xs"], np.float32)
    ei = np.asarray(inputs["edge_index"])
    ea = np.asarray(inputs["edge_attr"], np.float32)
    N = xs.shape[1]
    src = ei[0].astype(np.int64)
    dst = ei[1].astype(np.int64)
    loops = np.arange(N)
    srcA = np.concatenate([src, loops])
    dstA = np.concatenate([dst, loops])
    deg = np.bincount(dstA, minlength=N).astype(np.float32)
    dinv = np.where(deg > 0, deg ** -0.5, 0.0).astype(np.float32)

    def sig(x):
        return (1.0 / (1.0 + np.exp(-x))).astype(np.float32)

    W = np.asarray(inputs["init_w"], np.float32)
    w_ih = np.asarray(inputs["w_ih"], np.float32)
    w_hh = np.asarray(inputs["w_hh"], np.float32)
    b = (np.asarray(inputs["b_ih"], np.float32) + np.asarray(inputs["b_hh"], np.float32))
    h = np.zeros((64, 64), np.float32)
    c = np.zeros((64, 64), np.float32)
    for t in range(xs.shape[0]):
        gates = W.T @ w_ih.T + h @ w_hh.T + b
        i, f, g, o = np.split(gates, 4, axis=1)
        c = sig(f) * c + sig(i) * np.tanh(g)
        h = sig(o) * np.tanh(c)
        W = h.T.copy()

    y = dinv[:, None] * (xs[-1] @ W)
    S = np.zeros((N, 64), np.float32)
    np.add.at(S, dstA, y[srcA])
    xl = np.maximum(S, 0.0)
    mlp_w1 = np.asarray(inputs["mlp_w1"], np.float32)
    U = dinv[:, None] * (xl @ mlp_w1[:64])
    V = dinv[:, None] * (xl @ mlp_w1[64:128])
    Cc = ea @ mlp_w1[128:] + np.asarray(inputs["mlp_b1"], np.float32)
    hid = np.maximum(U[src] + V[dst] + Cc, 0.0)
    return (hid @ np.asarray(inputs["mlp_w2"], np.float32))[:, 0] + np.asarray(inputs["mlp_b2"], np.float32)[0]


# ------------------------------ kernel entry ------------------------------

_CACHE = {}


def kernel(**inputs):
    """Full-input EvolvingGNN kernel on 8 TRN2 NeuronCores."""
    N = int(inputs["xs"].shape[1])
    E = int(inputs["edge_index"].shape[1])
    cfg = Cfg(N, E)
    in_maps, static, meta = prep(inputs, cfg)
    key = (N, E, tuple(static["capT"].ravel()), tuple(static["capE"].ravel()))
    nc = _CACHE.get(key)
    if nc is None:
        nc = build(cfg, static)
        _CACHE[key] = nc
    r = run_bass_kernel_spmd(nc, in_maps, core_ids=list(range(C)))
    return unshard(r.results, meta, E)


# revision 44
# speedup vs baseline: 2.3622x; 1.0827x over previous
"""EvolvingGNN (gnn_message_passing) kernel for 8 Trainium2 NeuronCores.

Self-contained: takes the full inputs of setup_inputs(), returns the full
[E] float32 logits.  See kernel() at the bottom.

Structure:
  host: dinv-scaled x4d (so messages gather straight from the input and the
        device-side phase-1 y materialization disappears -- segment_sum
        commutes with the @W5 right-matmul, applied per window after
        aggregation); balanced dst relabeling node -> (core, window,
        partition) equalizes per-(core,window,chunk) message counts across
        cores (gather capacities are max-over-cores); edge bucketing by
        (dst-core, dst-window, src-chunk) for P2 and (src-chunk, dst-chunk)
        cells for P4, with UV rows in a partition-major permuted layout so
        the UV_shard DMA writes are per-partition contiguous.
  P0: LSTM -> W5 (tiny).
  P2: gather x4d[src] (dma_gather spread over all 4 SWDGE queues -- the
      per-queue descriptor pacing is the bottleneck resource), cast bf16 on
      ScalarE, one-hot (iota==dl) scatter-add via TensorE into PSUM, then
      xl^T = relu(W5^T @ Sx) per window, kept in SBUF.
  P3: per window U||V = dinv * (xl @ [W1a|W1b]) bf16 packed in 256B rows,
      single AllGather bitcast to f32 (halves collective element count).
  P4: per edge: gather UV[src], UV[dst] (256B rows, greedy queue balance),
      U+V presummed on DVE, + ea@W1cb in PSUM, hid = relu(.),
      logits = hid @ w2 + b2.  First-superslab index tiles are preloaded
      before the collective (everything issued after it waits on it).
"""


import heapq

import numpy as np
import ml_dtypes

import concourse.bacc as bacc
import concourse.tile as tile
from concourse import mybir
from concourse.bass_utils import run_bass_kernel_spmd

F32 = mybir.dt.float32
BF16 = mybir.dt.bfloat16
I16 = mybir.dt.int16
AF = mybir.ActivationFunctionType
OP = mybir.AluOpType
BF_NP = np.dtype(ml_dtypes.bfloat16)

D = 64   # input/lstm dim
H = 64   # hidden dim
FE = 16  # edge feature dim
C = 8    # cores
NQ = 4   # SWDGE queues


class Cfg:
    def __init__(self, N, E, gw=5, ssl=32, sl=8):
        self.N, self.E = N, E
        self.NP = -(-N // (C * 128)) * C * 128      # padded nodes
        self.SHARD = self.NP // C                    # nodes per core
        self.WPC = self.SHARD // 128                 # windows per core
        self.CS = self.NP // 4                       # gather chunk rows
        assert self.CS < 32768, "chunk exceeds int16 gather range"
        assert E % C == 0
        self.EPC = E // C                            # edges per core
        self.GW = gw                                 # windows per ph2 gather group
        self.SSL = ssl                               # ph4 superslab tiles
        self.SL = sl                                 # ph4 inner slab tiles
        self.groups = [range(s, min(s + gw, self.WPC))
                       for s in range(0, self.WPC, gw)]


def _wrap16(arr_i16):
    """[S] -> [128, S/16] wrapped in 16 partitions, replicated x8."""
    w = arr_i16.reshape(-1, 16).T  # [16, S/16]
    return np.ascontiguousarray(np.tile(w, (8, 1)))


def _ranks_after_sort(sorted_keys):
    """rank of each element within its (contiguous) key run."""
    n = len(sorted_keys)
    if n == 0:
        return np.zeros(0, np.int64)
    change = np.r_[True, sorted_keys[1:] != sorted_keys[:-1]]
    starts = np.where(change)[0]
    return np.arange(n) - np.repeat(starts, np.diff(np.r_[starts, n]))


def prep(inputs, cfg):
    """Host preprocessing. Returns (in_maps, static, meta)."""
    N, E, NP, CS = cfg.N, cfg.E, cfg.NP, cfg.CS
    WPC, SHARD = cfg.WPC, cfg.SHARD

    ei = np.asarray(inputs["edge_index"])
    src = ei[0].astype(np.int64)
    dst = ei[1].astype(np.int64)
    loops = np.arange(N, dtype=np.int64)
    srcA = np.concatenate([src, loops])
    dstA = np.concatenate([dst, loops])

    deg = np.bincount(dstA, minlength=NP).astype(np.float32)
    deg[deg == 0] = 1.0
    dinv = (deg ** -0.5).astype(np.float32)

    # dinv-scaled input features (message payload), padded, ORIGINAL node order
    xs = np.asarray(inputs["xs"], dtype=np.float32)
    x4d = np.zeros((NP, D), np.float32)
    x4d[:N] = dinv[:N, None] * xs[-1]

    q = srcA // CS

    # ---- balanced dst relabeling: node -> (core, window, partition) so that
    # per-(core,window,chunk) message counts are even across cores (the
    # gather capacities are max-over-cores; balance cuts padded slots ~13%)
    tot = np.bincount(dstA, minlength=NP)
    order_n = np.argsort(-tot, kind="stable")
    NB = C * WPC
    heap = [(0, b) for b in range(NB)]
    heapq.heapify(heap)
    bucket_of = np.zeros(NP, np.int64)
    p_of = np.zeros(NP, np.int64)
    fill = np.zeros(NB, np.int64)
    for n in order_n:
        while True:
            load, b = heapq.heappop(heap)
            if fill[b] < 128:
                break
        bucket_of[n] = b
        p_of[n] = fill[b]
        fill[b] += 1
        if fill[b] < 128:
            heapq.heappush(heap, (load + int(tot[n]), b))
    core_of = bucket_of // WPC
    win_of = bucket_of % WPC
    # UV row of node n (partition-major within its shard)
    uvrow_of = core_of * SHARD + p_of * WPC + win_of
    dinv_sh_arr = np.zeros(NB * 128, np.float32)
    dinv_sh_arr[bucket_of * 128 + p_of] = dinv
    dinv_shard = [np.ascontiguousarray(
        dinv_sh_arr.reshape(C, WPC, 128)[c].T) for c in range(C)]

    # ---- phase 2 bucketing: key = (core, window, chunk) ----
    core = core_of[dstA]
    win = win_of[dstA]
    dloc = p_of[dstA].astype(np.float32)
    sloc = (srcA % CS).astype(np.int16)

    key = (core * WPC + win) * 4 + q
    order = np.argsort(key, kind="stable")
    skey = key[order]
    counts = np.bincount(key, minlength=C * WPC * 4).reshape(C, WPC, 4)
    capT = np.ceil(counts.max(axis=0) / 128).astype(np.int64)      # [WPC, 4] tiles

    # gather-stream order: (group, chunk, window) ; dl-stream order: (window, chunk)
    slots_wq = capT * 128
    g_off = np.zeros((WPC, 4), np.int64)     # slot offset in gather stream
    t_off = np.zeros((WPC, 4), np.int64)     # tile offset in dl stream
    pos = 0
    for g in cfg.groups:
        for qq in range(4):
            for w in g:
                g_off[w, qq] = pos
                pos += slots_wq[w, qq]
    total_slots2 = pos
    pos = 0
    for w in range(WPC):
        for qq in range(4):
            t_off[w, qq] = pos
            pos += capT[w, qq]
    total_tiles2 = pos

    ranks = _ranks_after_sort(skey)
    gpos = g_off[win[order], q[order]] + ranks                     # slot in gather stream
    dpos = t_off[win[order], q[order]] * 128 + ranks               # slot in dl stream
    corder = core[order]

    gidx_list, dl_list = [], []
    for c in range(C):
        m = corder == c
        garr = np.zeros(total_slots2, np.int16)
        garr[gpos[m]] = sloc[order][m]
        dlarr = np.full(total_tiles2 * 128, -1.0, np.float32)
        dlarr[dpos[m]] = dloc[order][m]
        gidx_list.append(_wrap16(garr))
        dl_list.append(np.ascontiguousarray(dlarr.reshape(-1, 128).T.astype(BF_NP)))

    # ---- phase 4 bucketing: per core (positional edge shard), key = (a, b) ----
    # UV rows live in the relabeled partition-major layout (uvrow_of)
    EPC = cfg.EPC
    perm_row = uvrow_of[src]
    perm_col = uvrow_of[dst]
    a_all = perm_row // CS
    b_all = perm_col // CS
    sl4 = (perm_row % CS).astype(np.int16)
    dl4 = (perm_col % CS).astype(np.int16)

    counts4 = np.zeros((C, 16), np.int64)
    key4 = (a_all * 4 + b_all)
    for c in range(C):
        counts4[c] = np.bincount(key4[c * EPC:(c + 1) * EPC], minlength=16)
    capE = np.ceil(counts4.max(axis=0) / 128).astype(np.int64)     # [16] tiles
    cell_off = np.r_[0, np.cumsum(capE)][:16]                       # tile offsets
    total_tiles4 = int(capE.sum())
    total_slots4 = total_tiles4 * 128

    ea = np.asarray(inputs["edge_attr"], dtype=np.float32)
    sidx_list, didx_list, eat_list, origmap = [], [], [], []
    for c in range(C):
        sl_ = slice(c * EPC, (c + 1) * EPC)
        k = key4[sl_]
        o = np.argsort(k, kind="stable")
        r = _ranks_after_sort(k[o])
        slotpos = cell_off[k[o]] * 128 + r
        sarr = np.zeros(total_slots4, np.int16)
        darr = np.zeros(total_slots4, np.int16)
        eat = np.zeros((17, total_slots4), np.float32)
        om = np.full(total_slots4, -1, np.int64)
        eids = np.arange(c * EPC, (c + 1) * EPC)[o]
        sarr[slotpos] = sl4[sl_][o]
        darr[slotpos] = dl4[sl_][o]
        eat[:16, slotpos] = ea[eids].T
        eat[16, slotpos] = 1.0
        om[slotpos] = eids
        sidx_list.append(_wrap16(sarr))
        didx_list.append(_wrap16(darr))
        eat_list.append(np.ascontiguousarray(eat.astype(BF_NP)))
        origmap.append(om)

    # ---- dense weights ----
    w_ihT = np.ascontiguousarray(np.asarray(inputs["w_ih"], np.float32).T)   # [64, 256]
    w_hhT = np.ascontiguousarray(np.asarray(inputs["w_hh"], np.float32).T)
    bstack = np.stack([np.asarray(inputs["b_ih"], np.float32),
                       np.asarray(inputs["b_hh"], np.float32)])              # [2, 256]
    ones2 = np.ones((2, D), np.float32)
    init_w = np.asarray(inputs["init_w"], np.float32)
    mlp_w1 = np.asarray(inputs["mlp_w1"], np.float32)
    W1ab = np.ascontiguousarray(
        np.hstack([mlp_w1[:H], mlp_w1[H:2 * H]]).astype(BF_NP))              # [64, 128]
    W1cb = np.ascontiguousarray(
        np.vstack([mlp_w1[2 * H:],
                   np.asarray(inputs["mlp_b1"], np.float32)[None]]).astype(BF_NP))  # [17, 64]
    w2b = np.ascontiguousarray(
        np.tile(np.asarray(inputs["mlp_w2"], np.float32).T, (128, 1)).astype(BF_NP))  # [128, 64]
    b2b = np.full((128, 1), np.asarray(inputs["mlp_b2"], np.float32)[0], np.float32)

    in_maps = []
    for c in range(C):
        in_maps.append(dict(
            x4d=x4d, dinvS=dinv_shard[c],
            gidx=gidx_list[c], dl_all=dl_list[c],
            sidx=sidx_list[c], didx=didx_list[c], ea_t=eat_list[c],
            w_ihT=w_ihT, w_hhT=w_hhT, bstack=bstack, ones2=ones2,
            init_w=init_w, W1ab=W1ab, W1cb=W1cb, w2b=w2b, b2b=b2b,
        ))

    static = dict(capT=capT, g_off=g_off, t_off=t_off,
                  total_slots2=total_slots2, total_tiles2=total_tiles2,
                  capE=capE, cell_off=cell_off,
                  total_slots4=total_slots4, total_tiles4=total_tiles4)
    meta = dict(origmap=origmap, total_tiles4=total_tiles4)
    return in_maps, static, meta


def unshard(results, meta, E):
    logits = np.zeros(E, np.float32)
    for c in range(C):
        out = np.asarray(results[c]["logits_out"])  # [128, T4]
        flat = out.T.reshape(-1)                    # slot j*128+p at [p, j] -> flat[j*128+p]
        om = meta["origmap"][c]
        m = om >= 0
        logits[om[m]] = flat[m]
    return logits


def build(cfg, static):
    NP, CS, WPC, SHARD = cfg.NP, cfg.CS, cfg.WPC, cfg.SHARD
    capT, g_off, t_off = static["capT"], static["g_off"], static["t_off"]
    S2, T2 = static["total_slots2"], static["total_tiles2"]
    capE, cell_off = static["capE"], static["cell_off"]
    S4, T4 = static["total_slots4"], static["total_tiles4"]

    nc = bacc.Bacc("TRN2", target_bir_lowering=False, num_devices=C,
                   num_swdge_queues=NQ)

    # ---- params ----
    P = lambda name, shape, dt=F32: nc.declare_dram_parameter(name, list(shape), dt, isOutput=False)
    x4d = P("x4d", [NP, D])
    dinvS = P("dinvS", [128, WPC])
    gidx = P("gidx", [128, S2 // 16], I16)
    dl_all = P("dl_all", [128, T2], BF16)
    sidx = P("sidx", [128, S4 // 16], I16)
    didx = P("didx", [128, S4 // 16], I16)
    ea_t = P("ea_t", [17, S4], BF16)
    w_ihT = P("w_ihT", [D, 4 * D])
    w_hhT = P("w_hhT", [D, 4 * D])
    bstack = P("bstack", [2, 4 * D])
    ones2 = P("ones2", [2, D])
    init_w = P("init_w", [D, H])
    W1ab = P("W1ab", [H, 2 * H], BF16)
    W1cb = P("W1cb", [FE + 1, H], BF16)
    w2b = P("w2b", [128, H], BF16)
    b2b = P("b2b", [128, 1])
    logits_out = nc.declare_dram_parameter("logits_out", [128, T4], F32, isOutput=True)

    # ---- internal dram ----
    # UV is bf16 [*, 2H] data stored/collected as f32 [*, H] (pure bitcast):
    # halves the AllGather element count and keeps 256B gather elements.
    UV_shard = nc.dram_tensor("UV_shard", [SHARD, H], F32)
    UV_full = nc.dram_tensor("UV_full", [NP, H], F32, addr_space="Shared")

    iota_np = np.tile(np.arange(128, dtype=np.float32), (128, 1))
    iota_dram = nc.inline_tensor(iota_np, name="iota128")
    ident_dram = nc.inline_tensor(np.eye(128, dtype=np.float32), name="ident128")

    with tile.TileContext(nc) as tc:
        with tc.tile_pool(name="persist", bufs=1) as pp:
            iota_sb = pp.tile([128, 128], F32)
            nc.sync.dma_start(out=iota_sb[:], in_=iota_dram[:])
            iotab_sb = pp.tile([128, 128], BF16)
            nc.vector.tensor_copy(out=iotab_sb[:], in_=iota_sb[:])
            ident_sb = pp.tile([128, 128], F32)
            nc.sync.dma_start(out=ident_sb[:], in_=ident_dram[:])
            identb_sb = pp.tile([128, 128], BF16)
            nc.vector.tensor_copy(out=identb_sb[:], in_=ident_sb[:])
            w2b_sb = pp.tile([128, H], BF16)
            nc.sync.dma_start(out=w2b_sb[:], in_=w2b[:])
            b2b_sb = pp.tile([128, 1], F32)
            nc.sync.dma_start(out=b2b_sb[:], in_=b2b[:])
            W1ab_sb = pp.tile([H, 2 * H], BF16)
            nc.sync.dma_start(out=W1ab_sb[:], in_=W1ab[:])
            W1cb_sb = pp.tile([FE + 1, H], BF16)
            nc.sync.dma_start(out=W1cb_sb[:], in_=W1cb[:])
            dinv_sh = pp.tile([128, WPC], F32)
            nc.sync.dma_start(out=dinv_sh[:], in_=dinvS[:])
            # xl^T of this core's shard, bf16, lives in SBUF for all of P2/P3
            xts = pp.tile([H, SHARD], BF16)

            # ---- phase 0: LSTM -> W5 in w_sb [D, H]; bf16 copy in w5b ----
            w_sb = pp.tile([D, H], F32)
            w5b = pp.tile([D, H], BF16)
            nc.sync.dma_start(out=w_sb[:], in_=init_w[:])
            with (
                tc.tile_pool(name="lstm", bufs=1) as lp,
                tc.tile_pool(name="lstm_ps", bufs=2, space="PSUM") as lps,
            ):
                wih_sb = lp.tile([D, 4 * D], F32)
                nc.sync.dma_start(out=wih_sb[:], in_=w_ihT[:])
                whh_sb = lp.tile([D, 4 * D], F32)
                nc.sync.dma_start(out=whh_sb[:], in_=w_hhT[:])
                bst_sb = lp.tile([2, 4 * D], F32)
                nc.sync.dma_start(out=bst_sb[:], in_=bstack[:])
                one2_sb = lp.tile([2, D], F32)
                nc.sync.dma_start(out=one2_sb[:], in_=ones2[:])
                c_sb = lp.tile([D, D], F32)
                nc.vector.memset(c_sb[:], 0)

                for t in range(5):
                    pg = lps.tile([D, 4 * D], F32, space="PSUM", tag="pg")
                    nc.tensor.matmul(out=pg[:], lhsT=w_sb[:], rhs=wih_sb[:],
                                     start=True, stop=False)
                    if t > 0:
                        nc.tensor.matmul(out=pg[:], lhsT=w_sb[:], rhs=whh_sb[:],
                                         start=False, stop=False)
                    nc.tensor.matmul(out=pg[:], lhsT=one2_sb[:], rhs=bst_sb[:],
                                     start=False, stop=True)
                    sif = lp.tile([D, 2 * D], F32, tag="sif")
                    nc.scalar.activation(out=sif[:], in_=pg[:, 0:2 * D], func=AF.Sigmoid)
                    gt = lp.tile([D, D], F32, tag="gt")
                    nc.scalar.activation(out=gt[:], in_=pg[:, 2 * D:3 * D], func=AF.Tanh)
                    ot = lp.tile([D, D], F32, tag="ot")
                    nc.scalar.activation(out=ot[:], in_=pg[:, 3 * D:4 * D], func=AF.Sigmoid)
                    t1 = lp.tile([D, D], F32, tag="t1")
                    nc.vector.tensor_tensor(out=t1[:], in0=sif[:, 0:D], in1=gt[:], op=OP.mult)
                    nc.vector.tensor_tensor(out=c_sb[:], in0=sif[:, D:2 * D], in1=c_sb[:], op=OP.mult)
                    nc.vector.tensor_tensor(out=c_sb[:], in0=c_sb[:], in1=t1[:], op=OP.add)
                    th = lp.tile([D, D], F32, tag="th")
                    nc.scalar.activation(out=th[:], in_=c_sb[:], func=AF.Tanh)
                    hh = lp.tile([D, D], F32, tag="hh")
                    nc.vector.tensor_tensor(out=hh[:], in0=ot[:], in1=th[:], op=OP.mult)
                    pw = lps.tile([D, D], F32, space="PSUM", tag="pw")
                    nc.tensor.transpose(out=pw[:], in_=hh[:], identity=ident_sb[:D, :D])
                    nc.vector.tensor_copy(out=w_sb[:], in_=pw[:])
                nc.vector.tensor_copy(out=w5b[:], in_=w_sb[:])

            # ---- phase 2: gather x4d[src], one-hot scatter, W5 fold, relu ----
            with (
                tc.tile_pool(name="p2m", bufs=3) as p2m,
                tc.tile_pool(name="p2c", bufs=3) as p2c,
                tc.tile_pool(name="p2o", bufs=4) as p2o,
                tc.tile_pool(name="p2ps", bufs=2, space="PSUM") as p2ps,
                tc.tile_pool(name="p3", bufs=3) as p3,
                tc.tile_pool(name="p3ps", bufs=2, space="PSUM") as p3ps,
            ):
                for gi, g in enumerate(cfg.groups):
                    glist = list(g)
                    # gathers per chunk, then bf16 cast on (idle) ScalarE --
                    # also frees the fp32 gather buffer quickly
                    mb = []
                    for qq in range(4):
                        J = int(capT[glist, qq].sum())
                        mb.append(None)
                        if J == 0:
                            continue
                        buf = p2m.tile([128, J, H], F32, tag=f"mb{qq}")
                        base = int(g_off[glist[0], qq])
                        gix = p2m.tile([128, J * 8], I16, tag=f"gix{qq}")
                        nc.sync.dma_start(out=gix[:],
                                          in_=gidx[:, base // 16:(base + J * 128) // 16])
                        nc.gpsimd.dma_gather(
                            out_ap=buf[:],
                            in_ap=x4d[qq * CS:(qq + 1) * CS, :],
                            idxs_ap=gix[:],
                            num_idxs=J * 128,
                            num_idxs_reg=J * 128,
                            elem_size=H,
                            queue_num=qq % NQ, single_packet=False,
                        )
                        bufb = p2c.tile([128, J, H], BF16, tag=f"mbb{qq}")
                        nc.scalar.activation(out=bufb[:], in_=buf[:], func=AF.Copy)
                        mb[qq] = bufb
                    # dl slab
                    Tg = int(capT[glist, :].sum())
                    dl_sb = p2o.tile([128, Tg], BF16, tag="dl")
                    tg0 = int(t_off[glist[0], 0])
                    nc.sync.dma_start(out=dl_sb[:], in_=dl_all[:, tg0:tg0 + Tg])
                    for wi, w in enumerate(glist):
                        Tw = int(capT[w, :].sum())
                        if Tw == 0:
                            continue
                        oh = p2o.tile([128, Tw, 128], BF16, tag="oh")
                        dloff = int(t_off[w, 0]) - tg0
                        nc.vector.tensor_tensor(
                            out=oh[:],
                            in0=iotab_sb[:, None, :].broadcast_to([128, Tw, 128]),
                            in1=dl_sb[:, dloff:dloff + Tw, None].broadcast_to([128, Tw, 128]),
                            op=OP.is_equal)
                        pz = p2ps.tile([H, 128], F32, space="PSUM", tag="pz")
                        n_mm = 0
                        for qq in range(4):
                            nt = int(capT[w, qq])
                            if nt == 0:
                                continue
                            mcol = int(g_off[w, qq] - g_off[glist[0], qq]) // 128
                            ohcol = int(t_off[w, qq]) - tg0 - dloff
                            for j in range(nt):
                                nc.tensor.matmul(
                                    out=pz[:],
                                    lhsT=mb[qq][:, mcol + j, :],
                                    rhs=oh[:, ohcol + j, :],
                                    start=(n_mm == 0), stop=(n_mm == Tw - 1))
                                n_mm += 1
                        # Sx -> SBUF (bf16), then xl^T = relu(W5^T @ Sx)
                        sxs = p2o.tile([H, 128], BF16, tag="sxs")
                        nc.vector.tensor_copy(out=sxs[:], in_=pz[:])
                        pxl = p2ps.tile([H, 128], F32, space="PSUM", tag="pxl")
                        nc.tensor.matmul(out=pxl[:], lhsT=w5b[:], rhs=sxs[:],
                                         start=True, stop=True)
                        nc.scalar.activation(out=xts[:, w * 128:(w + 1) * 128],
                                             in_=pxl[:], func=AF.Relu)
                    # ---- phase 3 for this group's windows (issued eagerly so
                    # the UV writes and the AllGather can start early) ----
                    nb = len(glist)
                    uvw = p3.tile([128, cfg.GW, 2 * H], BF16, tag="uvw")
                    for k, w in enumerate(glist):
                        puv = p3ps.tile([128, 2 * H], F32, space="PSUM", tag="puv")
                        nc.tensor.matmul(out=puv[:],
                                         lhsT=xts[:, w * 128:(w + 1) * 128],
                                         rhs=W1ab_sb[:], start=True, stop=True)
                        nc.vector.tensor_scalar(
                            out=uvw[:, k, :], in0=puv[:],
                            scalar1=dinv_sh[:, w:w + 1], scalar2=None, op0=OP.mult)
                    # node (p, w) -> UV_shard row p*WPC + w  (contiguous per partition)
                    nc.sync.dma_start(
                        out=UV_shard.rearrange("(p W) f -> p W f", p=128)[:, glist[0]:glist[0] + nb, :],
                        in_=uvw[:, :nb, :].bitcast(F32))

            # ---- preload phase-4 first-superslab index tiles (before the
            # collective: everything issued after it waits for it) ----
            SSL, SL = cfg.SSL, cfg.SL
            with (
                tc.tile_pool(name="p4i", bufs=1) as p4i,
                tc.tile_pool(name="p4g", bufs=4) as p4g,
                tc.tile_pool(name="p4e", bufs=3) as p4e,
                tc.tile_pool(name="p4h", bufs=2) as p4h,
                tc.tile_pool(name="p4ps", bufs=4, space="PSUM") as p4ps,
            ):
                pre_six, pre_dix = {}, {}
                for cell in range(16):
                    nt0 = min(SSL, int(capE[cell]))
                    if nt0 <= 0:
                        continue
                    base = int(cell_off[cell]) * 128
                    t6 = p4i.tile([128, SSL * 8], I16, tag=f"p6_{cell}")
                    nc.sync.dma_start(out=t6[:, :nt0 * 8],
                                      in_=sidx[:, base // 16:(base + nt0 * 128) // 16])
                    t7 = p4i.tile([128, SSL * 8], I16, tag=f"p7_{cell}")
                    nc.sync.dma_start(out=t7[:, :nt0 * 8],
                                      in_=didx[:, base // 16:(base + nt0 * 128) // 16])
                    pre_six[cell], pre_dix[cell] = t6, t7

                # ---- allgather UV ----
                nc.gpsimd.collective_compute(
                    "AllGather", OP.bypass,
                    replica_groups=[list(range(C))],
                    ins=[UV_shard[:]], outs=[UV_full[:]])

                # ---- phase 4: edge MLP ----
                lg_sb = pp.tile([128, T4], F32)
                qload = [0, 0, 0, 0]  # tiles assigned per queue (greedy balance)

                def pick_q(nt):
                    q = min(range(NQ), key=lambda i: qload[i])
                    qload[q] += nt
                    return q
                for cell in range(16):
                    a, b = cell // 4, cell % 4
                    ctiles = int(capE[cell])
                    coff = int(cell_off[cell])
                    for s0 in range(0, ctiles, SSL):
                        nt = min(SSL, ctiles - s0)
                        base = (coff + s0) * 128
                        if s0 == 0:
                            six, dix = pre_six[cell], pre_dix[cell]
                        else:
                            six = p4g.tile([128, SSL * 8], I16, tag="six")
                            nc.sync.dma_start(out=six[:, :nt * 8],
                                              in_=sidx[:, base // 16:(base + nt * 128) // 16])
                            dix = p4g.tile([128, SSL * 8], I16, tag="dix")
                            nc.sync.dma_start(out=dix[:, :nt * 8],
                                              in_=didx[:, base // 16:(base + nt * 128) // 16])
                        usb = p4g.tile([128, SSL, H], F32, tag="usb")
                        nc.gpsimd.dma_gather(
                            out_ap=usb[:, :nt, :],
                            in_ap=UV_full[a * CS:(a + 1) * CS, :],
                            idxs_ap=six[:, :nt * 8],
                            num_idxs=nt * 128, num_idxs_reg=nt * 128,
                            elem_size=H, queue_num=pick_q(nt), single_packet=False)
                        vsb = p4g.tile([128, SSL, H], F32, tag="vsb")
                        nc.gpsimd.dma_gather(
                            out_ap=vsb[:, :nt, :],
                            in_ap=UV_full[b * CS:(b + 1) * CS, :],
                            idxs_ap=dix[:, :nt * 8],
                            num_idxs=nt * 128, num_idxs_reg=nt * 128,
                            elem_size=H, queue_num=pick_q(nt), single_packet=False)
                        easb = p4e.tile([FE + 1, SSL * 128], BF16, tag="easb")
                        nc.sync.dma_start(out=easb[:, :nt * 128],
                                          in_=ea_t[:, base:base + nt * 128])
                        # U[src] + V[dst] presum on DVE (bf16 views of f32 rows)
                        uvsum = p4e.tile([128, SSL, H], BF16, tag="uvsum")
                        nc.vector.tensor_tensor(
                            out=uvsum[:, :nt, :],
                            in0=usb[:, :nt, 0:H // 2].bitcast(BF16),
                            in1=vsb[:, :nt, H // 2:H].bitcast(BF16), op=OP.add)
                        for t0 in range(0, nt, SL):
                            nsl = min(SL, nt - t0)
                            ph = p4ps.tile([128, SL * H], F32, space="PSUM", tag="ph")
                            nc.tensor.matmul(
                                out=ph[:, :nsl * H],
                                lhsT=identb_sb[:],
                                rhs=uvsum[:, t0:t0 + nsl, :].rearrange("p t h -> p (t h)"),
                                start=True, stop=False)
                            for t in range(nsl):
                                nc.tensor.matmul(
                                    out=ph[:, t * H:(t + 1) * H],
                                    lhsT=easb[:, (t0 + t) * 128:(t0 + t + 1) * 128],
                                    rhs=W1cb_sb[:],
                                    start=False, stop=(t == nsl - 1))
                            hid = p4h.tile([128, SL, H], BF16, tag="hid")
                            nc.scalar.activation(
                                out=hid[:, :nsl, :],
                                in_=ph[:, :nsl * H].rearrange("p (t h) -> p t h", t=nsl),
                                func=AF.Relu)
                            prod = p4h.tile([128, SL, H], F32, tag="prod")
                            nc.vector.tensor_tensor(
                                out=prod[:, :nsl, :], in0=hid[:, :nsl, :],
                                in1=w2b_sb[:, None, :].broadcast_to([128, nsl, H]),
                                op=OP.mult)
                            nc.vector.tensor_reduce(
                                out=lg_sb[:, coff + s0 + t0:coff + s0 + t0 + nsl],
                                in_=prod[:, :nsl, :],
                                axis=mybir.AxisListType.X, op=OP.add)
                # + b2, write out
                nc.vector.tensor_scalar(
                    out=lg_sb[:], in0=lg_sb[:], scalar1=b2b_sb[:, 0:1], scalar2=None,
                    op0=OP.add)
                nc.sync.dma_start(out=logits_out[:], in_=lg_sb[:])

    nc.compile()
    return nc


# ---------------- numpy reference (mirrors the jax reference) ----------------

def numpy_ref(inputs):
    xs = np.asarray(inputs["xs"], np.float32)
    ei = np.asarray(inputs["edge_index"])
    ea = np.asarray(inputs["edge_attr"], np.float32)
    N = xs.shape[1]
    src = ei[0].astype(np.int64)
    dst = ei[1].astype(np.int64)
    loops = np.arange(N)
    srcA = np.concatenate([src, loops])
    dstA = np.concatenate([dst, loops])
    deg = np.bincount(dstA, minlength=N).astype(np.float32)
    dinv = np.where(deg > 0, deg ** -0.5, 0.0).astype(np.float32)

    def sig(x):
        return (1.0 / (1.0 + np.exp(-x))).astype(np.float32)

    W = np.asarray(inputs["init_w"], np.float32)
    w_ih = np.asarray(inputs["w_ih"], np.float32)
    w_hh = np.asarray(inputs["w_hh"], np.float32)
    b = (np.asarray(inputs["b_ih"], np.float32) + np.asarray(inputs["b_hh"], np.float32))
    h = np.zeros((64, 64), np.float32)
    c = np.zeros((64, 64), np.float32)
    for t in range(xs.shape[0]):
        gates = W.T @ w_ih.T + h @ w_hh.T + b
        i, f, g, o = np.split(gates, 4, axis=1)
        c = sig(f) * c + sig(i) * np.tanh(g)
        h = sig(o) * np.tanh(c)
        W = h.T.copy()

    y = dinv[:, None] * (xs[-1] @ W)
    S = np.zeros((N, 64), np.float32)
    np.add.at(S, dstA, y[srcA])
    xl = np.maximum(S, 0.0)
    mlp_w1 = np.asarray(inputs["mlp_w1"], np.float32)
    U = dinv[:, None] * (xl @ mlp_w1[:64])
    V = dinv[:, None] * (xl @ mlp_w1[64:128])
    Cc = ea @ mlp_w1[128:] + np.asarray(inputs["mlp_b1"], np.float32)
    hid = np.maximum(U[src] + V[dst] + Cc, 0.0)
    return (hid @ np.asarray(inputs["mlp_w2"], np.float32))[:, 0] + np.asarray(inputs["mlp_b2"], np.float32)[0]


# ------------------------------ kernel entry ------------------------------

_CACHE = {}


def kernel(**inputs):
    """Full-input EvolvingGNN kernel on 8 TRN2 NeuronCores."""
    N = int(inputs["xs"].shape[1])
    E = int(inputs["edge_index"].shape[1])
    cfg = Cfg(N, E)
    in_maps, static, meta = prep(inputs, cfg)
    key = (N, E, tuple(static["capT"].ravel()), tuple(static["capE"].ravel()))
    nc = _CACHE.get(key)
    if nc is None:
        nc = build(cfg, static)
        _CACHE[key] = nc
    r = run_bass_kernel_spmd(nc, in_maps, core_ids=list(range(C)))
    return unshard(r.results, meta, E)
